# revision 22
# baseline (speedup 1.0000x reference)
"""Trainium2 Bass kernel for the FGWF objective:

    out = sum(cost_mat(graph, graph_b, prob, prob_b, tran, embedding, embedding_b) * tran)

Decomposition (all heavy terms on device, final O(N) dots on host in f64):
  sum(cost*T) = sum_i (f1_i + 0.5*||E_i||^2) * rowsum(T)_i
              + sum_j (f2_j + 0.5*||Eb_j||^2) * colsum(T)_j
              - 2 * <G @ T, T @ Gb>              (two 2048^3 matmuls, bf16)
              - <E, T @ Eb>                      (feature cross term)
  with f1 = (G^2) @ p_s, f2 = (Gb^2) @ p_t.

Sharding over 8 NeuronCores: 2D grid (4 row-blocks x 2 col-blocks) for the
main matmuls; rank-1 terms sharded by rows/cols/embedding-half so every core
runs the identical SPMD program on different data.
"""
import numpy as np
import ml_dtypes

import concourse.bass as bass
from concourse import mybir
from concourse import bass_utils
from concourse.tile import TileContext

BF16 = ml_dtypes.bfloat16
F32 = mybir.dt.float32
BF = mybir.dt.bfloat16
N = 2048
NCORES = 8

_cache = {}


def _split_waits(nc):
    """The walrus build here rejects >1 sem-wait per instruction; hoist extra
    waits onto preceding same-engine nops."""
    MAXW = 1
    for fn in nc.m.functions:
        for b in fn.blocks:
            out = []
            changed = False
            for inst in b.instructions:
                si = inst.sync_info
                waits = list(si.on_wait) if si and si.on_wait else []
                if len(waits) > MAXW:
                    changed = True
                    k = 0
                    while len(waits) > MAXW:
                        chunk, waits = waits[:MAXW], waits[MAXW:]
                        out.append(mybir.InstNoOp(
                            name=f"{inst.name}-wsplit{k}", engine=inst.engine,
                            sync_info=mybir.SyncInfo(on_wait=chunk, on_update=[]),
                            bass_nofuse=True))
                        k += 1
                    inst.sync_info = mybir.SyncInfo(
                        on_wait=waits,
                        on_update=list(si.on_update) if si.on_update else [])
                out.append(inst)
            if changed:
                b.instructions = out


def _build():
    nc = bass.Bass()
    AL = mybir.AluOpType

    # per-core inputs (host-sharded)
    csT = nc.declare_dram_parameter("csT", [N, 512], BF, isOutput=False)
    tT = nc.declare_dram_parameter("tT", [N, 512], BF, isOutput=False)
    tcc = nc.declare_dram_parameter("tcc", [N, 1024], BF, isOutput=False)
    gbc = nc.declare_dram_parameter("gbc", [N, 1024], BF, isOutput=False)
    gbT = nc.declare_dram_parameter("gbT", [N, 256], F8, isOutput=False)
    ebf = nc.declare_dram_parameter("ebf", [N, 64], BF, isOutput=False)
    efT = nc.declare_dram_parameter("efT", [64, 512], F32, isOutput=False)
    ebs = nc.declare_dram_parameter("ebs", [256, 128], F32, isOutput=False)
    psb = nc.declare_dram_parameter("psb", [N, 16], F8, isOutput=False)
    ptb = nc.declare_dram_parameter("ptb", [N, 16], F8, isOutput=False)

    # outputs
    ocols_d = nc.declare_dram_parameter("out_cols", [128, 26], F32, isOutput=True)
    orows_d = nc.declare_dram_parameter("out_rows", [1, 1792], F32, isOutput=True)
    oemb_d = nc.declare_dram_parameter("out_emb", [64, 1], F32, isOutput=True)

    with TileContext(nc) as tc:
        with (
            tc.tile_pool(name="big", bufs=1) as big,
            tc.tile_pool(name="asb", bufs=8) as asb_p,
            tc.tile_pool(name="sqa", bufs=2) as sqa_p,
            tc.tile_pool(name="sqb", bufs=2) as sqb_p,
            tc.tile_pool(name="tout", bufs=2) as tout_p,
            tc.tile_pool(name="pa", bufs=4, space="PSUM") as pa_p,
            tc.tile_pool(name="pb", bufs=2, space="PSUM") as pb_p,
            tc.tile_pool(name="pe", bufs=1, space="PSUM") as pe_p,
            tc.tile_pool(name="prow", bufs=1, space="PSUM") as prow_p,
        ):
            csT_sb = big.tile([128, 16, 512], BF, tag="csT")
            tT_sb = big.tile([128, 16, 512], BF, tag="tT")
            tcc_sb = big.tile([128, 16, 1024], BF, tag="tcc")
            gbc_sb = big.tile([128, 16, 1024], BF, tag="gbc")
            gbT_sb = big.tile([128, 16, 256], F8, tag="gbT")
            ebf_sb = big.tile([128, 16, 64], BF, tag="ebf")
            efT_sb = big.tile([64, 512], F32, tag="efT")
            ebs_sb = big.tile([128, 2, 128], F32, tag="ebs")
            psb_sb = big.tile([128, 16, 16], F8, tag="psb")
            ptb_sb = big.tile([128, 16, 16], F8, tag="ptb")
            ones64 = big.tile([64, 1], F32, tag="ones")
            ones128 = big.tile([128, 1], BF, tag="ones128")
            sqe_sb = big.tile([64, 512], F32, tag="sqe")
            sqall = big.tile([128, 16, 512], F8, tag="sqall")
            sqball = big.tile([128, 16, 256], F8, tag="sqball")
            toute_sb = big.tile([64, 512], F32, tag="toute")
            ocols = big.tile([128, 26], F32, tag="ocols")
            orows = big.tile([1, 1792], F32, tag="orows")
            oemb = big.tile([64, 1], F32, tag="oemb")

            csT_ap = csT.rearrange("(p t) w -> p t w", p=128)
            tT_ap = tT.rearrange("(p t) w -> p t w", p=128)
            tcc_ap = tcc.rearrange("(p t) w -> p t w", p=128)
            gbc_ap = gbc.rearrange("(p t) w -> p t w", p=128)
            gbT_ap = gbT.rearrange("(p t) w -> p t w", p=128)
            ebf_ap = ebf.rearrange("(p t) w -> p t w", p=128)

            nc.vector.memset(ones64[:], 1.0)
            nc.vector.memset(ones128[:], 1.0)
            warm_sb = big.tile([128, 512], BF, tag="warm")
            nc.gpsimd.memset(warm_sb[:], 0.0)
            # PE warmup: keep TensorE busy from t~1us so HAM un-throttles
            # before the first data-dependent matmuls; results are discarded.
            pw = prow_p.tile([1, 512], F32, tag="prow")
            for w in range(12):
                nc.tensor.matmul(pw[:1, :], warm_sb[:, 0:1], warm_sb[:],
                                 start=True, stop=True, skip_group_check=True)
            # A-operands first (PE starts on these), then B, then misc
            nc.sync.dma_start(out=psb_sb[:], in_=psb.rearrange("(p t) w -> p t w", p=128))
            nc.sync.dma_start(out=ptb_sb[:], in_=ptb.rearrange("(p t) w -> p t w", p=128))
            nc.sync.dma_start(out=ebf_sb[:], in_=ebf_ap[:])
            for ks in (slice(0, 2), slice(2, 4), slice(4, 6), slice(6, 8),
                       slice(8, 12), slice(12, 16)):
                nc.sync.dma_start(out=csT_sb[:, ks, :], in_=csT_ap[:, ks, :])
                nc.sync.dma_start(out=tcc_sb[:, ks, :], in_=tcc_ap[:, ks, :])
            for q in range(4):
                ks = slice(4 * q, 4 * q + 4)
                nc.sync.dma_start(out=tT_sb[:, ks, :], in_=tT_ap[:, ks, :])
                nc.sync.dma_start(out=gbc_sb[:, ks, :], in_=gbc_ap[:, ks, :])
            nc.sync.dma_start(out=gbT_sb[:], in_=gbT_ap[:])
            nc.sync.dma_start(out=efT_sb[:], in_=efT[:])
            nc.sync.dma_start(out=ebs_sb[:], in_=ebs.rearrange("(p s) d -> p s d", p=128))

            # ---- A-phase: A[pair] = (G[rblk] @ T[:, ccols-half]) -> SBUF ----
            a_tiles = {}
            for n in range(2):
                for m in range(4):
                    msl = slice(128 * m, 128 * m + 128)
                    nsl = slice(512 * n, 512 * n + 512)
                    pa = pa_p.tile([128, 512], F32)
                    for q in range(8):
                        nc.tensor.matmul(pa[:], csT_sb[:, 2 * q:2 * q + 2, msl],
                                         tcc_sb[:, 2 * q:2 * q + 2, nsl],
                                         start=(q == 0), stop=(q == 7),
                                         perf_mode=mybir.MatmulPerfMode.DoubleRow)
                    ca = asb_p.tile([128, 512], F32)
                    nc.scalar.copy(ca[:], pa[:])
                    a_tiles[n * 4 + m] = ca

            # ---- precompute squares (ACT/DVE, fills engine idle time) ----
            for k in range(16):
                nc.scalar.activation(sqall[:, k, :], csT_sb[:, k, :],
                                     mybir.ActivationFunctionType.Square)
            for k in range(16):
                nc.vector.tensor_mul(sqball[:, k, :], gbT_sb[:, k, :], gbT_sb[:, k, :])

            # ---- f1 row [1,512]: sum_k p_s[k] * G[i,k]^2 ----
            pf1 = prow_p.tile([1, 512], F32, tag="prow")
            for q in range(8):
                nc.tensor.matmul(pf1[:], psb_sb[:, 2 * q:2 * q + 2, 0:1],
                                 sqall[:, 2 * q:2 * q + 2, :],
                                 start=(q == 0), stop=(q == 7),
                                 perf_mode=mybir.MatmulPerfMode.DoubleRow)
            nc.scalar.copy(orows[:1, 0:512], pf1[:])

            # ---- S_emb: psum_E = Eb_half^T @ T[rblk]^T = (T Eb)^T  [64, 512] ----
            pe_ = pe_p.tile([64, 512], F32)
            for j in range(16):
                nc.tensor.matmul(pe_[:], ebf_sb[:, j, :], tT_sb[:, j, :],
                                 start=(j == 0), stop=(j == 15))
            nc.vector.scalar_tensor_tensor(
                out=toute_sb[:], in0=efT_sb[:], scalar=1.0, in1=pe_[:],
                op0=AL.mult, op1=AL.mult, accum_out=oemb[:, 0:1])

            # ---- rowsum(T) for rblk via PE: ones^T contraction over j ----
            pr = prow_p.tile([1, 512], F32, tag="prow")
            for q in range(8):
                nc.tensor.matmul(pr[:], ones128[:, :, 0:1],
                                 tT_sb[:, 2 * q:2 * q + 2, :],
                                 start=(q == 0), stop=(q == 7),
                                 perf_mode=mybir.MatmulPerfMode.DoubleRow)
            nc.scalar.copy(orows[:1, 1280:1792], pr[:])

            # ---- f2 row [1,256]: sum_k p_t[k] * Gb[j,k]^2, j in jslice ----
            pf2 = prow_p.tile([1, 512], F32, tag="prow")
            for q in range(8):
                nc.tensor.matmul(pf2[:1, 0:256], ptb_sb[:, 2 * q:2 * q + 2, 0:1],
                                 sqball[:, 2 * q:2 * q + 2, :],
                                 start=(q == 0), stop=(q == 7),
                                 perf_mode=mybir.MatmulPerfMode.DoubleRow)
            nc.scalar.copy(orows[:1, 512:768], pf2[:1, 0:256])

            # ---- ||E_i||^2 (this d-half) row [1,512] ----
            nc.scalar.activation(sqe_sb[:], efT_sb[:], mybir.ActivationFunctionType.Square)
            pne = prow_p.tile([1, 512], F32, tag="prow")
            nc.tensor.matmul(pne[:], ones64[:], sqe_sb[:], start=True, stop=True)
            nc.scalar.copy(orows[:1, 768:1280], pne[:])

            # ---- ||Eb_j||^2 for jslice -> ocols[:, 24:26] ----
            for s2 in range(2):
                to = tout_p.tile([128, 512], F32)
                nc.scalar.activation(
                    to[:, 0:128], ebs_sb[:, s2, :],
                    mybir.ActivationFunctionType.Square,
                    accum_out=ocols[:, 24 + s2:25 + s2])

            # ---- colsum(T) partial over rblk (DVE free-reduce) -> ocols[:, 8:24] ----
            for t in range(16):
                nc.vector.reduce_sum(ocols[:, 8 + t:9 + t], tT_sb[:, t, :],
                                     axis=mybir.AxisListType.X)

            # ---- B-phase + fused <A,B> accumulation ----
            for n in range(2):
                for m in range(4):
                    msl = slice(128 * m, 128 * m + 128)
                    nsl = slice(512 * n, 512 * n + 512)
                    pb = pb_p.tile([128, 512], F32)
                    for q in range(8):
                        nc.tensor.matmul(pb[:], tT_sb[:, 2 * q:2 * q + 2, msl],
                                         gbc_sb[:, 2 * q:2 * q + 2, nsl],
                                         start=(q == 0), stop=(q == 7),
                                         perf_mode=mybir.MatmulPerfMode.DoubleRow)
                    to = tout_p.tile([128, 512], F32)
                    pair = n * 4 + m
                    nc.vector.scalar_tensor_tensor(
                        out=to[:], in0=a_tiles[pair][:], scalar=1.0, in1=pb[:],
                        op0=AL.mult, op1=AL.mult,
                        accum_out=ocols[:, pair:pair + 1])

            nc.sync.dma_start(out=ocols_d[:], in_=ocols[:])
            nc.sync.dma_start(out=orows_d[:], in_=orows[:1, :])
            nc.sync.dma_start(out=oemb_d[:], in_=oemb[:])

    _split_waits(nc)
    return nc


def _prep_inputs(graph, embedding, prob, graph_b, embedding_b, prob_b, tran):
    bf = lambda x: np.ascontiguousarray(x).astype(BF16)
    G = np.asarray(graph, np.float32)
    E = np.asarray(embedding, np.float32)
    P = np.asarray(prob, np.float32).reshape(N)
    GB = np.asarray(graph_b, np.float32)
    EB = np.asarray(embedding_b, np.float32)
    PB = np.asarray(prob_b, np.float32).reshape(N)
    T = np.asarray(tran, np.float32)

    psb = np.zeros((N, 16), FP8)
    psb[:, 0] = (P * 2048.0).astype(FP8)
    ptb = np.zeros((N, 16), FP8)
    ptb[:, 0] = (PB * 2048.0).astype(FP8)
    in_maps = []
    for idx in range(NCORES):
        r, c = idx // 2, idx % 2
        rblk = slice(512 * r, 512 * r + 512)
        ccols = slice(1024 * c, 1024 * c + 1024)
        dh = slice(64 * c, 64 * c + 64)
        jsl = slice(256 * idx, 256 * idx + 256)
        in_maps.append({
            "csT": bf(G[rblk, :].T),
            "tT": bf(T[rblk, :].T),
            "tcc": bf(T[:, ccols]),
            "gbc": bf(GB[:, ccols]),
            "gbT": f8(GB[jsl, :].T),
            "ebf": bf(EB[:, dh]),
            "efT": np.ascontiguousarray(E[rblk, dh].T, dtype=np.float32),
            "ebs": np.ascontiguousarray(EB[jsl, :], dtype=np.float32),
            "psb": psb,
            "ptb": ptb,
        })
    return in_maps


def _reduce(results):
    S_main = 0.0
    S_emb = 0.0
    f1 = np.zeros(N, np.float64)
    f2 = np.zeros(N, np.float64)
    r = np.zeros(N, np.float64)
    c = np.zeros(N, np.float64)
    nE = np.zeros(N, np.float64)
    nEB = np.zeros(N, np.float64)
    for idx in range(NCORES):
        rr, cc = idx // 2, idx % 2
        rblk = slice(512 * rr, 512 * rr + 512)
        jsl = slice(256 * idx, 256 * idx + 256)
        ocols = np.asarray(results[idx]["out_cols"], np.float64)
        orows = np.asarray(results[idx]["out_rows"], np.float64)[0]
        oemb = np.asarray(results[idx]["out_emb"], np.float64)
        S_main += ocols[:, 0:8].sum()
        S_emb += oemb.sum()
        f2[jsl] = orows[512:768] / 2048.0
        nEB[jsl] = ocols[:, 24:26].reshape(256)
        nE[rblk] += orows[768:1280]
        if cc == 0:
            f1[rblk] = orows[0:512] / 2048.0
            r[rblk] = orows[1280:1792]
            # colsum partial over rblk: j = 16*p + t
            c += ocols[:, 8:24].reshape(N)
    total = (
        ((f1 + 0.5 * nE) * r).sum()
        + ((f2 + 0.5 * nEB) * c).sum()
        - 2.0 * S_main
        - S_emb
    )
    return np.float32(total)


def run_spmd(in_maps, trace=False, **kw):
    if "nc" not in _cache:
        _cache["nc"] = _build()
    return bass_utils.run_bass_kernel_spmd(
        _cache["nc"], in_maps, list(range(NCORES)), trace=trace, **kw)


def kernel(graph, embedding, prob, graph_b, embedding_b, prob_b, tran,
           weights, ole_coeff, idx):
    in_maps = _prep_inputs(graph, embedding, prob, graph_b, embedding_b,
                           prob_b, tran)
    res = run_spmd(in_maps)
    return _reduce(res.results)


# revision 23
# speedup vs baseline: 1.0043x; 1.0043x over previous
"""Trainium2 Bass kernel for the FGWF objective:

    out = sum(cost_mat(graph, graph_b, prob, prob_b, tran, embedding, embedding_b) * tran)

Decomposition (all heavy terms on device, final O(N) dots on host in f64):
  sum(cost*T) = sum_i (f1_i + 0.5*||E_i||^2) * rowsum(T)_i
              + sum_j (f2_j + 0.5*||Eb_j||^2) * colsum(T)_j
              - 2 * <G @ T, T @ Gb>              (two 2048^3 matmuls, bf16)
              - <E, T @ Eb>                      (feature cross term)
  with f1 = (G^2) @ p_s, f2 = (Gb^2) @ p_t.

Sharding over 8 NeuronCores: 2D grid (4 row-blocks x 2 col-blocks) for the
main matmuls; rank-1 terms sharded by rows/cols/embedding-half so every core
runs the identical SPMD program on different data.
"""
import numpy as np
import ml_dtypes

import concourse.bass as bass
from concourse import mybir
from concourse import bass_utils
from concourse.tile import TileContext

BF16 = ml_dtypes.bfloat16
F32 = mybir.dt.float32
BF = mybir.dt.bfloat16
N = 2048
NCORES = 8

_cache = {}


def _split_waits(nc):
    """The walrus build here rejects >1 sem-wait per instruction; hoist extra
    waits onto preceding same-engine nops."""
    MAXW = 1
    for fn in nc.m.functions:
        for b in fn.blocks:
            out = []
            changed = False
            for inst in b.instructions:
                si = inst.sync_info
                waits = list(si.on_wait) if si and si.on_wait else []
                if len(waits) > MAXW:
                    changed = True
                    k = 0
                    while len(waits) > MAXW:
                        chunk, waits = waits[:MAXW], waits[MAXW:]
                        out.append(mybir.InstNoOp(
                            name=f"{inst.name}-wsplit{k}", engine=inst.engine,
                            sync_info=mybir.SyncInfo(on_wait=chunk, on_update=[]),
                            bass_nofuse=True))
                        k += 1
                    inst.sync_info = mybir.SyncInfo(
                        on_wait=waits,
                        on_update=list(si.on_update) if si.on_update else [])
                out.append(inst)
            if changed:
                b.instructions = out


def _build():
    nc = bass.Bass()
    AL = mybir.AluOpType

    # per-core inputs (host-sharded)
    csT = nc.declare_dram_parameter("csT", [N, 512], BF, isOutput=False)
    tT = nc.declare_dram_parameter("tT", [N, 512], BF, isOutput=False)
    tcc = nc.declare_dram_parameter("tcc", [N, 1024], BF, isOutput=False)
    gbc = nc.declare_dram_parameter("gbc", [N, 1024], BF, isOutput=False)
    gbT = nc.declare_dram_parameter("gbT", [N, 256], F8, isOutput=False)
    ebf = nc.declare_dram_parameter("ebf", [N, 64], BF, isOutput=False)
    efT = nc.declare_dram_parameter("efT", [64, 512], F32, isOutput=False)
    ebs = nc.declare_dram_parameter("ebs", [256, 128], F32, isOutput=False)
    psb = nc.declare_dram_parameter("psb", [N, 16], F8, isOutput=False)
    ptb = nc.declare_dram_parameter("ptb", [N, 16], F8, isOutput=False)

    # outputs
    ocols_d = nc.declare_dram_parameter("out_cols", [128, 26], F32, isOutput=True)
    orows_d = nc.declare_dram_parameter("out_rows", [1, 1792], F32, isOutput=True)
    oemb_d = nc.declare_dram_parameter("out_emb", [64, 1], F32, isOutput=True)

    with TileContext(nc) as tc:
        with (
            tc.tile_pool(name="big", bufs=1) as big,
            tc.tile_pool(name="asb", bufs=8) as asb_p,
            tc.tile_pool(name="sqa", bufs=2) as sqa_p,
            tc.tile_pool(name="sqb", bufs=2) as sqb_p,
            tc.tile_pool(name="tout", bufs=2) as tout_p,
            tc.tile_pool(name="pa", bufs=4, space="PSUM") as pa_p,
            tc.tile_pool(name="pb", bufs=2, space="PSUM") as pb_p,
            tc.tile_pool(name="pe", bufs=1, space="PSUM") as pe_p,
            tc.tile_pool(name="prow", bufs=1, space="PSUM") as prow_p,
        ):
            csT_sb = big.tile([128, 16, 512], BF, tag="csT")
            tT_sb = big.tile([128, 16, 512], BF, tag="tT")
            tcc_sb = big.tile([128, 16, 1024], BF, tag="tcc")
            gbc_sb = big.tile([128, 16, 1024], BF, tag="gbc")
            gbT_sb = big.tile([128, 16, 256], F8, tag="gbT")
            ebf_sb = big.tile([128, 16, 64], BF, tag="ebf")
            efT_sb = big.tile([64, 512], F32, tag="efT")
            ebs_sb = big.tile([128, 2, 128], F32, tag="ebs")
            psb_sb = big.tile([128, 16, 16], F8, tag="psb")
            ptb_sb = big.tile([128, 16, 16], F8, tag="ptb")
            ones64 = big.tile([64, 1], F32, tag="ones")
            ones128 = big.tile([128, 1], BF, tag="ones128")
            sqe_sb = big.tile([64, 512], F32, tag="sqe")
            sqall = big.tile([128, 16, 512], F8, tag="sqall")
            sqball = big.tile([128, 16, 256], F8, tag="sqball")
            toute_sb = big.tile([64, 512], F32, tag="toute")
            ocols = big.tile([128, 26], F32, tag="ocols")
            orows = big.tile([1, 1792], F32, tag="orows")
            oemb = big.tile([64, 1], F32, tag="oemb")

            csT_ap = csT.rearrange("(p t) w -> p t w", p=128)
            tT_ap = tT.rearrange("(p t) w -> p t w", p=128)
            tcc_ap = tcc.rearrange("(p t) w -> p t w", p=128)
            gbc_ap = gbc.rearrange("(p t) w -> p t w", p=128)
            gbT_ap = gbT.rearrange("(p t) w -> p t w", p=128)
            ebf_ap = ebf.rearrange("(p t) w -> p t w", p=128)

            nc.vector.memset(ones64[:], 1.0)
            nc.vector.memset(ones128[:], 1.0)
            warm_sb = big.tile([128, 512], BF, tag="warm")
            nc.gpsimd.memset(warm_sb[:], 0.0)
            # PE warmup: keep TensorE busy from t~1us so HAM un-throttles
            # before the first data-dependent matmuls; results are discarded.
            pw = prow_p.tile([1, 512], F32, tag="prow")
            for w in range(16):
                nc.tensor.matmul(pw[:1, :], warm_sb[:, 0:1], warm_sb[:],
                                 start=True, stop=True, skip_group_check=True)
            # A-operands first (PE starts on these), then B, then misc
            nc.sync.dma_start(out=psb_sb[:], in_=psb.rearrange("(p t) w -> p t w", p=128))
            nc.sync.dma_start(out=ptb_sb[:], in_=ptb.rearrange("(p t) w -> p t w", p=128))
            nc.sync.dma_start(out=ebf_sb[:], in_=ebf_ap[:])
            for ks in (slice(0, 2), slice(2, 4), slice(4, 6), slice(6, 8),
                       slice(8, 12), slice(12, 16)):
                nc.sync.dma_start(out=csT_sb[:, ks, :], in_=csT_ap[:, ks, :])
                nc.sync.dma_start(out=tcc_sb[:, ks, :], in_=tcc_ap[:, ks, :])
            for q in range(4):
                ks = slice(4 * q, 4 * q + 4)
                nc.sync.dma_start(out=tT_sb[:, ks, :], in_=tT_ap[:, ks, :])
                nc.sync.dma_start(out=gbc_sb[:, ks, :], in_=gbc_ap[:, ks, :])
            nc.sync.dma_start(out=gbT_sb[:], in_=gbT_ap[:])
            nc.sync.dma_start(out=efT_sb[:], in_=efT[:])
            nc.sync.dma_start(out=ebs_sb[:], in_=ebs.rearrange("(p s) d -> p s d", p=128))

            # ---- A-phase: A[pair] = (G[rblk] @ T[:, ccols-half]) -> SBUF ----
            a_tiles = {}
            for n in range(2):
                for m in range(4):
                    msl = slice(128 * m, 128 * m + 128)
                    nsl = slice(512 * n, 512 * n + 512)
                    pa = pa_p.tile([128, 512], F32)
                    for q in range(8):
                        nc.tensor.matmul(pa[:], csT_sb[:, 2 * q:2 * q + 2, msl],
                                         tcc_sb[:, 2 * q:2 * q + 2, nsl],
                                         start=(q == 0), stop=(q == 7),
                                         perf_mode=mybir.MatmulPerfMode.DoubleRow)
                    ca = asb_p.tile([128, 512], F32)
                    nc.scalar.copy(ca[:], pa[:])
                    a_tiles[n * 4 + m] = ca

            # ---- precompute squares (ACT/DVE, fills engine idle time) ----
            for k in range(16):
                nc.scalar.activation(sqall[:, k, :], csT_sb[:, k, :],
                                     mybir.ActivationFunctionType.Square)
            for k in range(16):
                nc.vector.tensor_mul(sqball[:, k, :], gbT_sb[:, k, :], gbT_sb[:, k, :])

            # ---- f1 row [1,512]: sum_k p_s[k] * G[i,k]^2 ----
            pf1 = prow_p.tile([1, 512], F32, tag="prow")
            for q in range(8):
                nc.tensor.matmul(pf1[:], psb_sb[:, 2 * q:2 * q + 2, 0:1],
                                 sqall[:, 2 * q:2 * q + 2, :],
                                 start=(q == 0), stop=(q == 7),
                                 perf_mode=mybir.MatmulPerfMode.DoubleRow)
            nc.scalar.copy(orows[:1, 0:512], pf1[:])

            # ---- S_emb: psum_E = Eb_half^T @ T[rblk]^T = (T Eb)^T  [64, 512] ----
            pe_ = pe_p.tile([64, 512], F32)
            for j in range(16):
                nc.tensor.matmul(pe_[:], ebf_sb[:, j, :], tT_sb[:, j, :],
                                 start=(j == 0), stop=(j == 15))
            nc.vector.scalar_tensor_tensor(
                out=toute_sb[:], in0=efT_sb[:], scalar=1.0, in1=pe_[:],
                op0=AL.mult, op1=AL.mult, accum_out=oemb[:, 0:1])

            # ---- rowsum(T) for rblk via PE: ones^T contraction over j ----
            pr = prow_p.tile([1, 512], F32, tag="prow")
            for q in range(8):
                nc.tensor.matmul(pr[:], ones128[:, :, 0:1],
                                 tT_sb[:, 2 * q:2 * q + 2, :],
                                 start=(q == 0), stop=(q == 7),
                                 perf_mode=mybir.MatmulPerfMode.DoubleRow)
            nc.scalar.copy(orows[:1, 1280:1792], pr[:])

            # ---- f2 row [1,256]: sum_k p_t[k] * Gb[j,k]^2, j in jslice ----
            pf2 = prow_p.tile([1, 512], F32, tag="prow")
            for q in range(8):
                nc.tensor.matmul(pf2[:1, 0:256], ptb_sb[:, 2 * q:2 * q + 2, 0:1],
                                 sqball[:, 2 * q:2 * q + 2, :],
                                 start=(q == 0), stop=(q == 7),
                                 perf_mode=mybir.MatmulPerfMode.DoubleRow)
            nc.scalar.copy(orows[:1, 512:768], pf2[:1, 0:256])

            # ---- ||E_i||^2 (this d-half) row [1,512] ----
            nc.scalar.activation(sqe_sb[:], efT_sb[:], mybir.ActivationFunctionType.Square)
            pne = prow_p.tile([1, 512], F32, tag="prow")
            nc.tensor.matmul(pne[:], ones64[:], sqe_sb[:], start=True, stop=True)
            nc.scalar.copy(orows[:1, 768:1280], pne[:])

            # ---- ||Eb_j||^2 for jslice -> ocols[:, 24:26] ----
            for s2 in range(2):
                to = tout_p.tile([128, 512], F32)
                nc.scalar.activation(
                    to[:, 0:128], ebs_sb[:, s2, :],
                    mybir.ActivationFunctionType.Square,
                    accum_out=ocols[:, 24 + s2:25 + s2])

            # ---- colsum(T) partial over rblk (DVE free-reduce) -> ocols[:, 8:24] ----
            for t in range(16):
                nc.vector.reduce_sum(ocols[:, 8 + t:9 + t], tT_sb[:, t, :],
                                     axis=mybir.AxisListType.X)

            # ---- B-phase + fused <A,B> accumulation ----
            for n in range(2):
                for m in range(4):
                    msl = slice(128 * m, 128 * m + 128)
                    nsl = slice(512 * n, 512 * n + 512)
                    pb = pb_p.tile([128, 512], F32)
                    for q in range(8):
                        nc.tensor.matmul(pb[:], tT_sb[:, 2 * q:2 * q + 2, msl],
                                         gbc_sb[:, 2 * q:2 * q + 2, nsl],
                                         start=(q == 0), stop=(q == 7),
                                         perf_mode=mybir.MatmulPerfMode.DoubleRow)
                    to = tout_p.tile([128, 512], F32)
                    pair = n * 4 + m
                    nc.vector.scalar_tensor_tensor(
                        out=to[:], in0=a_tiles[pair][:], scalar=1.0, in1=pb[:],
                        op0=AL.mult, op1=AL.mult,
                        accum_out=ocols[:, pair:pair + 1])

            nc.sync.dma_start(out=ocols_d[:], in_=ocols[:])
            nc.sync.dma_start(out=orows_d[:], in_=orows[:1, :])
            nc.sync.dma_start(out=oemb_d[:], in_=oemb[:])

    _split_waits(nc)
    return nc


def _prep_inputs(graph, embedding, prob, graph_b, embedding_b, prob_b, tran):
    bf = lambda x: np.ascontiguousarray(x).astype(BF16)
    G = np.asarray(graph, np.float32)
    E = np.asarray(embedding, np.float32)
    P = np.asarray(prob, np.float32).reshape(N)
    GB = np.asarray(graph_b, np.float32)
    EB = np.asarray(embedding_b, np.float32)
    PB = np.asarray(prob_b, np.float32).reshape(N)
    T = np.asarray(tran, np.float32)

    psb = np.zeros((N, 16), FP8)
    psb[:, 0] = (P * 2048.0).astype(FP8)
    ptb = np.zeros((N, 16), FP8)
    ptb[:, 0] = (PB * 2048.0).astype(FP8)
    in_maps = []
    for idx in range(NCORES):
        r, c = idx // 2, idx % 2
        rblk = slice(512 * r, 512 * r + 512)
        ccols = slice(1024 * c, 1024 * c + 1024)
        dh = slice(64 * c, 64 * c + 64)
        jsl = slice(256 * idx, 256 * idx + 256)
        in_maps.append({
            "csT": bf(G[rblk, :].T),
            "tT": bf(T[rblk, :].T),
            "tcc": bf(T[:, ccols]),
            "gbc": bf(GB[:, ccols]),
            "gbT": f8(GB[jsl, :].T),
            "ebf": bf(EB[:, dh]),
            "efT": np.ascontiguousarray(E[rblk, dh].T, dtype=np.float32),
            "ebs": np.ascontiguousarray(EB[jsl, :], dtype=np.float32),
            "psb": psb,
            "ptb": ptb,
        })
    return in_maps


def _reduce(results):
    S_main = 0.0
    S_emb = 0.0
    f1 = np.zeros(N, np.float64)
    f2 = np.zeros(N, np.float64)
    r = np.zeros(N, np.float64)
    c = np.zeros(N, np.float64)
    nE = np.zeros(N, np.float64)
    nEB = np.zeros(N, np.float64)
    for idx in range(NCORES):
        rr, cc = idx // 2, idx % 2
        rblk = slice(512 * rr, 512 * rr + 512)
        jsl = slice(256 * idx, 256 * idx + 256)
        ocols = np.asarray(results[idx]["out_cols"], np.float64)
        orows = np.asarray(results[idx]["out_rows"], np.float64)[0]
        oemb = np.asarray(results[idx]["out_emb"], np.float64)
        S_main += ocols[:, 0:8].sum()
        S_emb += oemb.sum()
        f2[jsl] = orows[512:768] / 2048.0
        nEB[jsl] = ocols[:, 24:26].reshape(256)
        nE[rblk] += orows[768:1280]
        if cc == 0:
            f1[rblk] = orows[0:512] / 2048.0
            r[rblk] = orows[1280:1792]
            # colsum partial over rblk: j = 16*p + t
            c += ocols[:, 8:24].reshape(N)
    total = (
        ((f1 + 0.5 * nE) * r).sum()
        + ((f2 + 0.5 * nEB) * c).sum()
        - 2.0 * S_main
        - S_emb
    )
    return np.float32(total)


def run_spmd(in_maps, trace=False, **kw):
    if "nc" not in _cache:
        _cache["nc"] = _build()
    return bass_utils.run_bass_kernel_spmd(
        _cache["nc"], in_maps, list(range(NCORES)), trace=trace, **kw)


def kernel(graph, embedding, prob, graph_b, embedding_b, prob_b, tran,
           weights, ole_coeff, idx):
    in_maps = _prep_inputs(graph, embedding, prob, graph_b, embedding_b,
                           prob_b, tran)
    res = run_spmd(in_maps)
    return _reduce(res.results)


# revision 25
# speedup vs baseline: 1.0180x; 1.0136x over previous
"""Trainium2 Bass kernel for the FGWF objective:

    out = sum(cost_mat(graph, graph_b, prob, prob_b, tran, embedding, embedding_b) * tran)

Decomposition (all heavy terms on device, final O(N) dots on host in f64):
  sum(cost*T) = sum_i (f1_i + 0.5*||E_i||^2) * rowsum(T)_i
              + sum_j (f2_j + 0.5*||Eb_j||^2) * colsum(T)_j
              - 2 * <G @ T, T @ Gb>              (two 2048^3 matmuls, fp8 DoubleRow)
              - <E, T @ Eb>                      (feature cross term)
  with f1 = (G^2) @ p_s, f2 = (Gb^2) @ p_t.

Sharding over 8 NeuronCores: 2D grid (4 row-blocks x 2 col-blocks) for the
main matmuls; rank-1 terms sharded by rows/cols/embedding-half so every core
runs the identical SPMD program on different data. tran is pre-scaled by 2^20
(and prob/prob_b by 2048) on the host so fp8-e4m3 avoids subnormal flush;
partial outputs are rescaled during the host-side reduction.
"""
import numpy as np
import ml_dtypes

import concourse.bass as bass
from concourse import mybir
from concourse import bass_utils
from concourse.tile import TileContext

BF16 = ml_dtypes.bfloat16
FP8 = ml_dtypes.float8_e4m3
F32 = mybir.dt.float32
BF = mybir.dt.bfloat16
F8 = mybir.dt.float8e4
TSCALE = 2.0 ** 20
N = 2048
NCORES = 8

_cache = {}


def _split_waits(nc):
    """The walrus build here rejects >1 sem-wait per instruction; hoist extra
    waits onto preceding same-engine nops."""
    MAXW = 1
    for fn in nc.m.functions:
        for b in fn.blocks:
            out = []
            changed = False
            for inst in b.instructions:
                si = inst.sync_info
                waits = list(si.on_wait) if si and si.on_wait else []
                if len(waits) > MAXW:
                    changed = True
                    k = 0
                    while len(waits) > MAXW:
                        chunk, waits = waits[:MAXW], waits[MAXW:]
                        out.append(mybir.InstNoOp(
                            name=f"{inst.name}-wsplit{k}", engine=inst.engine,
                            sync_info=mybir.SyncInfo(on_wait=chunk, on_update=[]),
                            bass_nofuse=True))
                        k += 1
                    inst.sync_info = mybir.SyncInfo(
                        on_wait=waits,
                        on_update=list(si.on_update) if si.on_update else [])
                out.append(inst)
            if changed:
                b.instructions = out


def _build():
    nc = bass.Bass()
    AL = mybir.AluOpType

    # per-core inputs (host-sharded)
    csT = nc.declare_dram_parameter("csT", [N, 512], F8, isOutput=False)
    tT = nc.declare_dram_parameter("tT", [N, 512], F8, isOutput=False)
    tcc = nc.declare_dram_parameter("tcc", [N, 1024], F8, isOutput=False)
    gbc = nc.declare_dram_parameter("gbc", [N, 1024], F8, isOutput=False)
    gbT = nc.declare_dram_parameter("gbT", [N, 256], F8, isOutput=False)
    ebf = nc.declare_dram_parameter("ebf", [N, 64], F8, isOutput=False)
    efT = nc.declare_dram_parameter("efT", [64, 512], F32, isOutput=False)
    ebs = nc.declare_dram_parameter("ebs", [256, 128], F32, isOutput=False)
    psb = nc.declare_dram_parameter("psb", [N, 16], F8, isOutput=False)
    ptb = nc.declare_dram_parameter("ptb", [N, 16], F8, isOutput=False)

    # outputs
    ocols_d = nc.declare_dram_parameter("out_cols", [128, 26], F32, isOutput=True)
    orows_d = nc.declare_dram_parameter("out_rows", [1, 1792], F32, isOutput=True)
    oemb_d = nc.declare_dram_parameter("out_emb", [64, 1], F32, isOutput=True)

    with TileContext(nc) as tc:
        with (
            tc.tile_pool(name="big", bufs=1) as big,
            tc.tile_pool(name="asb", bufs=8) as asb_p,
            tc.tile_pool(name="sqa", bufs=2) as sqa_p,
            tc.tile_pool(name="sqb", bufs=2) as sqb_p,
            tc.tile_pool(name="tout", bufs=2) as tout_p,
            tc.tile_pool(name="pa", bufs=4, space="PSUM") as pa_p,
            tc.tile_pool(name="pb", bufs=2, space="PSUM") as pb_p,
            tc.tile_pool(name="pe", bufs=1, space="PSUM") as pe_p,
            tc.tile_pool(name="prow", bufs=1, space="PSUM") as prow_p,
        ):
            csT_sb = big.tile([128, 16, 512], F8, tag="csT")
            tT_sb = big.tile([128, 16, 512], F8, tag="tT")
            tcc_sb = big.tile([128, 16, 1024], F8, tag="tcc")
            gbc_sb = big.tile([128, 16, 1024], F8, tag="gbc")
            gbT_sb = big.tile([128, 16, 256], F8, tag="gbT")
            ebf_sb = big.tile([128, 16, 64], F8, tag="ebf")
            efT_sb = big.tile([64, 512], F32, tag="efT")
            ebs_sb = big.tile([128, 2, 128], F32, tag="ebs")
            psb_sb = big.tile([128, 16, 16], F8, tag="psb")
            ptb_sb = big.tile([128, 16, 16], F8, tag="ptb")
            ones64 = big.tile([64, 1], F32, tag="ones")
            ones128 = big.tile([128, 2, 16], F8, tag="ones128")
            sqe_sb = big.tile([64, 512], F32, tag="sqe")
            sqall = big.tile([128, 16, 512], F8, tag="sqall")
            sqball = big.tile([128, 16, 256], F8, tag="sqball")
            toute_sb = big.tile([64, 512], F32, tag="toute")
            ocols = big.tile([128, 26], F32, tag="ocols")
            orows = big.tile([1, 1792], F32, tag="orows")
            oemb = big.tile([64, 1], F32, tag="oemb")

            csT_ap = csT.rearrange("(p t) w -> p t w", p=128)
            tT_ap = tT.rearrange("(p t) w -> p t w", p=128)
            tcc_ap = tcc.rearrange("(p t) w -> p t w", p=128)
            gbc_ap = gbc.rearrange("(p t) w -> p t w", p=128)
            gbT_ap = gbT.rearrange("(p t) w -> p t w", p=128)
            ebf_ap = ebf.rearrange("(p t) w -> p t w", p=128)

            nc.vector.memset(ones64[:], 1.0)
            nc.vector.memset(ones128[:], 1.0)
            warm_sb = big.tile([128, 512], BF, tag="warm")
            nc.gpsimd.memset(warm_sb[:], 0.0)
            # PE warmup: keep TensorE busy from t~1us so HAM un-throttles
            # before the first data-dependent matmuls; results are discarded.
            pw = prow_p.tile([1, 512], F32, tag="prow")
            for w in range(16):
                nc.tensor.matmul(pw[:1, :], warm_sb[:, 0:1], warm_sb[:],
                                 start=True, stop=True, skip_group_check=True)
            # A-operands first (PE starts on these), then B, then misc
            nc.sync.dma_start(out=psb_sb[:], in_=psb.rearrange("(p t) w -> p t w", p=128))
            nc.sync.dma_start(out=ptb_sb[:], in_=ptb.rearrange("(p t) w -> p t w", p=128))
            nc.sync.dma_start(out=ebf_sb[:], in_=ebf_ap[:])
            for ks in (slice(0, 2), slice(2, 4), slice(4, 6), slice(6, 8),
                       slice(8, 12), slice(12, 16)):
                nc.sync.dma_start(out=csT_sb[:, ks, :], in_=csT_ap[:, ks, :])
                nc.sync.dma_start(out=tcc_sb[:, ks, :], in_=tcc_ap[:, ks, :])
            for q in range(4):
                ks = slice(4 * q, 4 * q + 4)
                nc.sync.dma_start(out=tT_sb[:, ks, :], in_=tT_ap[:, ks, :])
                nc.sync.dma_start(out=gbc_sb[:, ks, :], in_=gbc_ap[:, ks, :])
            nc.sync.dma_start(out=gbT_sb[:], in_=gbT_ap[:])
            nc.sync.dma_start(out=efT_sb[:], in_=efT[:])
            nc.sync.dma_start(out=ebs_sb[:], in_=ebs.rearrange("(p s) d -> p s d", p=128))

            # ---- A-phase: A[pair] = (G[rblk] @ T[:, ccols-half]) -> SBUF ----
            a_tiles = {}
            for n in range(2):
                for m in range(4):
                    msl = slice(128 * m, 128 * m + 128)
                    nsl = slice(512 * n, 512 * n + 512)
                    pa = pa_p.tile([128, 512], F32)
                    for q in range(8):
                        nc.tensor.matmul(pa[:], csT_sb[:, 2 * q:2 * q + 2, msl],
                                         tcc_sb[:, 2 * q:2 * q + 2, nsl],
                                         start=(q == 0), stop=(q == 7),
                                         perf_mode=mybir.MatmulPerfMode.DoubleRow)
                    ca = asb_p.tile([128, 512], F32)
                    nc.scalar.copy(ca[:], pa[:])
                    a_tiles[n * 4 + m] = ca

            # ---- precompute squares (ACT/DVE, fills engine idle time) ----
            for k in range(16):
                nc.scalar.activation(sqall[:, k, :], csT_sb[:, k, :],
                                     mybir.ActivationFunctionType.Square)
            for k in range(16):
                nc.vector.tensor_mul(sqball[:, k, :], gbT_sb[:, k, :], gbT_sb[:, k, :])

            # ---- f1 row [1,512]: sum_k p_s[k] * G[i,k]^2 ----
            pf1 = prow_p.tile([1, 512], F32, tag="prow")
            for q in range(8):
                nc.tensor.matmul(pf1[:], psb_sb[:, 2 * q:2 * q + 2, 0:1],
                                 sqall[:, 2 * q:2 * q + 2, :],
                                 start=(q == 0), stop=(q == 7),
                                 perf_mode=mybir.MatmulPerfMode.DoubleRow)
            nc.scalar.copy(orows[:1, 0:512], pf1[:])

            # ---- S_emb: psum_E = Eb_half^T @ T[rblk]^T = (T Eb)^T  [64, 512] ----
            pe_ = pe_p.tile([64, 512], F32)
            for q in range(8):
                nc.tensor.matmul(pe_[:], ebf_sb[:, 2 * q:2 * q + 2, :],
                                 tT_sb[:, 2 * q:2 * q + 2, :],
                                 start=(q == 0), stop=(q == 7),
                                 perf_mode=mybir.MatmulPerfMode.DoubleRow)
            nc.vector.scalar_tensor_tensor(
                out=toute_sb[:], in0=efT_sb[:], scalar=1.0, in1=pe_[:],
                op0=AL.mult, op1=AL.mult, accum_out=oemb[:, 0:1])

            # ---- rowsum(T) for rblk via PE: ones^T contraction over j ----
            pr = prow_p.tile([1, 512], F32, tag="prow")
            for q in range(8):
                nc.tensor.matmul(pr[:], ones128[:, :, 0:1],
                                 tT_sb[:, 2 * q:2 * q + 2, :],
                                 start=(q == 0), stop=(q == 7),
                                 perf_mode=mybir.MatmulPerfMode.DoubleRow)
            nc.scalar.copy(orows[:1, 1280:1792], pr[:])

            # ---- f2 row [1,256]: sum_k p_t[k] * Gb[j,k]^2, j in jslice ----
            pf2 = prow_p.tile([1, 512], F32, tag="prow")
            for q in range(8):
                nc.tensor.matmul(pf2[:1, 0:256], ptb_sb[:, 2 * q:2 * q + 2, 0:1],
                                 sqball[:, 2 * q:2 * q + 2, :],
                                 start=(q == 0), stop=(q == 7),
                                 perf_mode=mybir.MatmulPerfMode.DoubleRow)
            nc.scalar.copy(orows[:1, 512:768], pf2[:1, 0:256])

            # ---- ||E_i||^2 (this d-half) row [1,512] ----
            nc.scalar.activation(sqe_sb[:], efT_sb[:], mybir.ActivationFunctionType.Square)
            pne = prow_p.tile([1, 512], F32, tag="prow")
            nc.tensor.matmul(pne[:], ones64[:], sqe_sb[:], start=True, stop=True)
            nc.scalar.copy(orows[:1, 768:1280], pne[:])

            # ---- ||Eb_j||^2 for jslice -> ocols[:, 24:26] ----
            for s2 in range(2):
                to = tout_p.tile([128, 512], F32)
                nc.scalar.activation(
                    to[:, 0:128], ebs_sb[:, s2, :],
                    mybir.ActivationFunctionType.Square,
                    accum_out=ocols[:, 24 + s2:25 + s2])

            # ---- colsum(T) partial over rblk (DVE free-reduce) -> ocols[:, 8:24] ----
            for t in range(16):
                nc.vector.reduce_sum(ocols[:, 8 + t:9 + t], tT_sb[:, t, :],
                                     axis=mybir.AxisListType.X)

            # ---- B-phase + fused <A,B> accumulation ----
            for n in range(2):
                for m in range(4):
                    msl = slice(128 * m, 128 * m + 128)
                    nsl = slice(512 * n, 512 * n + 512)
                    pb = pb_p.tile([128, 512], F32)
                    for q in range(8):
                        nc.tensor.matmul(pb[:], tT_sb[:, 2 * q:2 * q + 2, msl],
                                         gbc_sb[:, 2 * q:2 * q + 2, nsl],
                                         start=(q == 0), stop=(q == 7),
                                         perf_mode=mybir.MatmulPerfMode.DoubleRow)
                    to = tout_p.tile([128, 512], F32)
                    pair = n * 4 + m
                    nc.vector.scalar_tensor_tensor(
                        out=to[:], in0=a_tiles[pair][:], scalar=1.0, in1=pb[:],
                        op0=AL.mult, op1=AL.mult,
                        accum_out=ocols[:, pair:pair + 1])

            nc.sync.dma_start(out=ocols_d[:], in_=ocols[:])
            nc.sync.dma_start(out=orows_d[:], in_=orows[:1, :])
            nc.sync.dma_start(out=oemb_d[:], in_=oemb[:])

    _split_waits(nc)
    return nc


def _prep_inputs(graph, embedding, prob, graph_b, embedding_b, prob_b, tran):
    G = np.asarray(graph, np.float32)
    E = np.asarray(embedding, np.float32)
    P = np.asarray(prob, np.float32).reshape(N)
    GB = np.asarray(graph_b, np.float32)
    EB = np.asarray(embedding_b, np.float32)
    PB = np.asarray(prob_b, np.float32).reshape(N)
    T = np.asarray(tran, np.float32)

    psb = np.zeros((N, 16), FP8)
    psb[:, 0] = (P * 2048.0).astype(FP8)
    ptb = np.zeros((N, 16), FP8)
    ptb[:, 0] = (PB * 2048.0).astype(FP8)
    in_maps = []
    for idx in range(NCORES):
        r, c = idx // 2, idx % 2
        rblk = slice(512 * r, 512 * r + 512)
        ccols = slice(1024 * c, 1024 * c + 1024)
        dh = slice(64 * c, 64 * c + 64)
        jsl = slice(256 * idx, 256 * idx + 256)
        f8 = lambda x: np.ascontiguousarray(x).astype(FP8)
        in_maps.append({
            "csT": f8(G[rblk, :].T),
            "tT": f8(T[rblk, :].T * TSCALE),
            "tcc": f8(T[:, ccols] * TSCALE),
            "gbc": f8(GB[:, ccols]),
            "gbT": f8(GB[jsl, :].T),
            "ebf": f8(EB[:, dh]),
            "efT": np.ascontiguousarray(E[rblk, dh].T, dtype=np.float32),
            "ebs": np.ascontiguousarray(EB[jsl, :], dtype=np.float32),
            "psb": psb,
            "ptb": ptb,
        })
    return in_maps


def _reduce(results):
    S_main = 0.0
    S_emb = 0.0
    f1 = np.zeros(N, np.float64)
    f2 = np.zeros(N, np.float64)
    r = np.zeros(N, np.float64)
    c = np.zeros(N, np.float64)
    nE = np.zeros(N, np.float64)
    nEB = np.zeros(N, np.float64)
    for idx in range(NCORES):
        rr, cc = idx // 2, idx % 2
        rblk = slice(512 * rr, 512 * rr + 512)
        jsl = slice(256 * idx, 256 * idx + 256)
        ocols = np.asarray(results[idx]["out_cols"], np.float64)
        orows = np.asarray(results[idx]["out_rows"], np.float64)[0]
        oemb = np.asarray(results[idx]["out_emb"], np.float64)
        S_main += ocols[:, 0:8].sum() / (TSCALE * TSCALE)
        S_emb += oemb.sum() / TSCALE
        f2[jsl] = orows[512:768] / 2048.0
        nEB[jsl] = ocols[:, 24:26].reshape(256)
        nE[rblk] += orows[768:1280]
        if cc == 0:
            f1[rblk] = orows[0:512] / 2048.0
            r[rblk] = orows[1280:1792] / TSCALE
            # colsum partial over rblk: j = 16*p + t
            c += ocols[:, 8:24].reshape(N) / TSCALE
    total = (
        ((f1 + 0.5 * nE) * r).sum()
        + ((f2 + 0.5 * nEB) * c).sum()
        - 2.0 * S_main
        - S_emb
    )
    return np.float32(total)


def run_spmd(in_maps, trace=False, **kw):
    if "nc" not in _cache:
        _cache["nc"] = _build()
    return bass_utils.run_bass_kernel_spmd(
        _cache["nc"], in_maps, list(range(NCORES)), trace=trace, **kw)


def kernel(graph, embedding, prob, graph_b, embedding_b, prob_b, tran,
           weights, ole_coeff, idx):
    in_maps = _prep_inputs(graph, embedding, prob, graph_b, embedding_b,
                           prob_b, tran)
    last_err = None
    for _attempt in range(3):
        try:
            res = run_spmd(in_maps)
            return _reduce(res.results)
        except Exception as e:  # transient NRT device errors seen under axon
            last_err = e
    raise last_err



# revision 26
# speedup vs baseline: 1.0327x; 1.0145x over previous
"""Trainium2 Bass kernel for the FGWF objective:

    out = sum(cost_mat(graph, graph_b, prob, prob_b, tran, embedding, embedding_b) * tran)

Decomposition (all heavy terms on device, final O(N) dots on host in f64):
  sum(cost*T) = sum_i (f1_i + 0.5*||E_i||^2) * rowsum(T)_i
              + sum_j (f2_j + 0.5*||Eb_j||^2) * colsum(T)_j
              - 2 * <G @ T, T @ Gb>              (two 2048^3 matmuls, fp8 DoubleRow)
              - <E, T @ Eb>                      (feature cross term)
  with f1 = (G^2) @ p_s, f2 = (Gb^2) @ p_t.

Sharding over 8 NeuronCores: 2D grid (4 row-blocks x 2 col-blocks) for the
main matmuls; rank-1 terms sharded by rows/cols/embedding-half so every core
runs the identical SPMD program on different data. tran is pre-scaled by 2^20
(and prob/prob_b by 2048) on the host so fp8-e4m3 avoids subnormal flush;
partial outputs are rescaled during the host-side reduction.
"""
import numpy as np
import ml_dtypes

import concourse.bass as bass
from concourse import mybir
from concourse import bass_utils
from concourse.tile import TileContext

BF16 = ml_dtypes.bfloat16
FP8 = ml_dtypes.float8_e4m3
F32 = mybir.dt.float32
BF = mybir.dt.bfloat16
F8 = mybir.dt.float8e4
TSCALE = 2.0 ** 20
N = 2048
NCORES = 8

_cache = {}


def _split_waits(nc):
    """The walrus build here rejects >1 sem-wait per instruction; hoist extra
    waits onto preceding same-engine nops."""
    MAXW = 1
    for fn in nc.m.functions:
        for b in fn.blocks:
            out = []
            changed = False
            for inst in b.instructions:
                si = inst.sync_info
                waits = list(si.on_wait) if si and si.on_wait else []
                if len(waits) > MAXW:
                    changed = True
                    k = 0
                    while len(waits) > MAXW:
                        chunk, waits = waits[:MAXW], waits[MAXW:]
                        out.append(mybir.InstNoOp(
                            name=f"{inst.name}-wsplit{k}", engine=inst.engine,
                            sync_info=mybir.SyncInfo(on_wait=chunk, on_update=[]),
                            bass_nofuse=True))
                        k += 1
                    inst.sync_info = mybir.SyncInfo(
                        on_wait=waits,
                        on_update=list(si.on_update) if si.on_update else [])
                out.append(inst)
            if changed:
                b.instructions = out


def _build():
    nc = bass.Bass()
    AL = mybir.AluOpType

    # per-core inputs (host-sharded)
    csT = nc.declare_dram_parameter("csT", [N, 512], F8, isOutput=False)
    tT = nc.declare_dram_parameter("tT", [N, 512], F8, isOutput=False)
    tcc = nc.declare_dram_parameter("tcc", [N, 1024], F8, isOutput=False)
    gbc = nc.declare_dram_parameter("gbc", [N, 1024], F8, isOutput=False)
    gbT = nc.declare_dram_parameter("gbT", [N, 256], F8, isOutput=False)
    ebf = nc.declare_dram_parameter("ebf", [N, 64], F8, isOutput=False)
    efT = nc.declare_dram_parameter("efT", [64, 512], F32, isOutput=False)
    ebs = nc.declare_dram_parameter("ebs", [256, 128], F32, isOutput=False)
    psb = nc.declare_dram_parameter("psb", [N, 16], F8, isOutput=False)
    ptb = nc.declare_dram_parameter("ptb", [N, 16], F8, isOutput=False)

    # outputs
    ocols_d = nc.declare_dram_parameter("out_cols", [128, 26], F32, isOutput=True)
    orows_d = nc.declare_dram_parameter("out_rows", [1, 1792], F32, isOutput=True)
    oemb_d = nc.declare_dram_parameter("out_emb", [64, 1], F32, isOutput=True)

    with TileContext(nc) as tc:
        with (
            tc.tile_pool(name="big", bufs=1) as big,
            tc.tile_pool(name="asb", bufs=8) as asb_p,
            tc.tile_pool(name="sqa", bufs=2) as sqa_p,
            tc.tile_pool(name="sqb", bufs=2) as sqb_p,
            tc.tile_pool(name="tout", bufs=2) as tout_p,
            tc.tile_pool(name="pa", bufs=4, space="PSUM") as pa_p,
            tc.tile_pool(name="pb", bufs=2, space="PSUM") as pb_p,
            tc.tile_pool(name="pe", bufs=1, space="PSUM") as pe_p,
            tc.tile_pool(name="prow", bufs=1, space="PSUM") as prow_p,
        ):
            csT_sb = big.tile([128, 16, 512], F8, tag="csT")
            tT_sb = big.tile([128, 16, 512], F8, tag="tT")
            tcc_sb = big.tile([128, 16, 1024], F8, tag="tcc")
            gbc_sb = big.tile([128, 16, 1024], F8, tag="gbc")
            gbT_sb = big.tile([128, 16, 256], F8, tag="gbT")
            ebf_sb = big.tile([128, 16, 64], F8, tag="ebf")
            efT_sb = big.tile([64, 512], F32, tag="efT")
            ebs_sb = big.tile([128, 2, 128], F32, tag="ebs")
            psb_sb = big.tile([128, 16, 16], F8, tag="psb")
            ptb_sb = big.tile([128, 16, 16], F8, tag="ptb")
            ones64 = big.tile([64, 1], F32, tag="ones")
            ones128 = big.tile([128, 2, 16], F8, tag="ones128")
            sqe_sb = big.tile([64, 512], F32, tag="sqe")
            sqall = big.tile([128, 16, 512], F8, tag="sqall")
            sqball = big.tile([128, 16, 256], F8, tag="sqball")
            toute_sb = big.tile([64, 512], F32, tag="toute")
            ocols = big.tile([128, 26], F32, tag="ocols")
            orows = big.tile([1, 1792], F32, tag="orows")
            oemb = big.tile([64, 1], F32, tag="oemb")

            csT_ap = csT.rearrange("(p t) w -> p t w", p=128)
            tT_ap = tT.rearrange("(p t) w -> p t w", p=128)
            tcc_ap = tcc.rearrange("(p t) w -> p t w", p=128)
            gbc_ap = gbc.rearrange("(p t) w -> p t w", p=128)
            gbT_ap = gbT.rearrange("(p t) w -> p t w", p=128)
            ebf_ap = ebf.rearrange("(p t) w -> p t w", p=128)

            nc.vector.memset(ones64[:], 1.0)
            nc.vector.memset(ones128[:], 1.0)
            warm_sb = big.tile([128, 512], BF, tag="warm")
            nc.gpsimd.memset(warm_sb[:], 0.0)
            # PE warmup: keep TensorE busy from t~1us so HAM un-throttles
            # before the first data-dependent matmuls; results are discarded.
            pw = prow_p.tile([1, 512], F32, tag="prow")
            for w in range(16):
                nc.tensor.matmul(pw[:1, :], warm_sb[:, 0:1], warm_sb[:],
                                 start=True, stop=True, skip_group_check=True)
            # A-operands first (PE starts on these), then B, then misc
            for ks in (slice(0, 2), slice(2, 4), slice(4, 6), slice(6, 8),
                       slice(8, 12), slice(12, 16)):
                nc.sync.dma_start(out=csT_sb[:, ks, :], in_=csT_ap[:, ks, :])
                nc.sync.dma_start(out=tcc_sb[:, ks, :], in_=tcc_ap[:, ks, :])
            nc.sync.dma_start(out=psb_sb[:], in_=psb.rearrange("(p t) w -> p t w", p=128))
            nc.sync.dma_start(out=ptb_sb[:], in_=ptb.rearrange("(p t) w -> p t w", p=128))
            nc.sync.dma_start(out=ebf_sb[:], in_=ebf_ap[:])
            for q in range(4):
                ks = slice(4 * q, 4 * q + 4)
                nc.sync.dma_start(out=tT_sb[:, ks, :], in_=tT_ap[:, ks, :])
                nc.sync.dma_start(out=gbc_sb[:, ks, :], in_=gbc_ap[:, ks, :])
            nc.sync.dma_start(out=gbT_sb[:], in_=gbT_ap[:])
            nc.sync.dma_start(out=efT_sb[:], in_=efT[:])
            nc.sync.dma_start(out=ebs_sb[:], in_=ebs.rearrange("(p s) d -> p s d", p=128))

            # ---- A-phase: A[pair] = (G[rblk] @ T[:, ccols-half]) -> SBUF ----
            a_tiles = {}
            for n in range(2):
                for m in range(4):
                    msl = slice(128 * m, 128 * m + 128)
                    nsl = slice(512 * n, 512 * n + 512)
                    pa = pa_p.tile([128, 512], F32)
                    for q in range(8):
                        nc.tensor.matmul(pa[:], csT_sb[:, 2 * q:2 * q + 2, msl],
                                         tcc_sb[:, 2 * q:2 * q + 2, nsl],
                                         start=(q == 0), stop=(q == 7),
                                         perf_mode=mybir.MatmulPerfMode.DoubleRow)
                    ca = asb_p.tile([128, 512], F32)
                    nc.scalar.copy(ca[:], pa[:])
                    a_tiles[n * 4 + m] = ca

            # ---- precompute squares (ACT/DVE, fills engine idle time) ----
            for k in range(16):
                nc.scalar.activation(sqall[:, k, :], csT_sb[:, k, :],
                                     mybir.ActivationFunctionType.Square)
            for k in range(16):
                nc.vector.tensor_mul(sqball[:, k, :], gbT_sb[:, k, :], gbT_sb[:, k, :])

            # ---- f1 row [1,512]: sum_k p_s[k] * G[i,k]^2 ----
            pf1 = prow_p.tile([1, 512], F32, tag="prow")
            for q in range(8):
                nc.tensor.matmul(pf1[:], psb_sb[:, 2 * q:2 * q + 2, 0:1],
                                 sqall[:, 2 * q:2 * q + 2, :],
                                 start=(q == 0), stop=(q == 7),
                                 perf_mode=mybir.MatmulPerfMode.DoubleRow)
            nc.scalar.copy(orows[:1, 0:512], pf1[:])

            # ---- S_emb: psum_E = Eb_half^T @ T[rblk]^T = (T Eb)^T  [64, 512] ----
            pe_ = pe_p.tile([64, 512], F32)
            for q in range(8):
                nc.tensor.matmul(pe_[:], ebf_sb[:, 2 * q:2 * q + 2, :],
                                 tT_sb[:, 2 * q:2 * q + 2, :],
                                 start=(q == 0), stop=(q == 7),
                                 perf_mode=mybir.MatmulPerfMode.DoubleRow)
            nc.vector.scalar_tensor_tensor(
                out=toute_sb[:], in0=efT_sb[:], scalar=1.0, in1=pe_[:],
                op0=AL.mult, op1=AL.mult, accum_out=oemb[:, 0:1])

            # ---- rowsum(T) for rblk via PE: ones^T contraction over j ----
            pr = prow_p.tile([1, 512], F32, tag="prow")
            for q in range(8):
                nc.tensor.matmul(pr[:], ones128[:, :, 0:1],
                                 tT_sb[:, 2 * q:2 * q + 2, :],
                                 start=(q == 0), stop=(q == 7),
                                 perf_mode=mybir.MatmulPerfMode.DoubleRow)
            nc.scalar.copy(orows[:1, 1280:1792], pr[:])

            # ---- f2 row [1,256]: sum_k p_t[k] * Gb[j,k]^2, j in jslice ----
            pf2 = prow_p.tile([1, 512], F32, tag="prow")
            for q in range(8):
                nc.tensor.matmul(pf2[:1, 0:256], ptb_sb[:, 2 * q:2 * q + 2, 0:1],
                                 sqball[:, 2 * q:2 * q + 2, :],
                                 start=(q == 0), stop=(q == 7),
                                 perf_mode=mybir.MatmulPerfMode.DoubleRow)
            nc.scalar.copy(orows[:1, 512:768], pf2[:1, 0:256])

            # ---- ||E_i||^2 (this d-half) row [1,512] ----
            nc.scalar.activation(sqe_sb[:], efT_sb[:], mybir.ActivationFunctionType.Square)
            pne = prow_p.tile([1, 512], F32, tag="prow")
            nc.tensor.matmul(pne[:], ones64[:], sqe_sb[:], start=True, stop=True)
            nc.scalar.copy(orows[:1, 768:1280], pne[:])

            # ---- ||Eb_j||^2 for jslice -> ocols[:, 24:26] ----
            for s2 in range(2):
                to = tout_p.tile([128, 512], F32)
                nc.scalar.activation(
                    to[:, 0:128], ebs_sb[:, s2, :],
                    mybir.ActivationFunctionType.Square,
                    accum_out=ocols[:, 24 + s2:25 + s2])

            # ---- colsum(T) partial over rblk (DVE free-reduce) -> ocols[:, 8:24] ----
            for t in range(16):
                nc.vector.reduce_sum(ocols[:, 8 + t:9 + t], tT_sb[:, t, :],
                                     axis=mybir.AxisListType.X)

            # ---- B-phase + fused <A,B> accumulation ----
            for n in range(2):
                for m in range(4):
                    msl = slice(128 * m, 128 * m + 128)
                    nsl = slice(512 * n, 512 * n + 512)
                    pb = pb_p.tile([128, 512], F32)
                    for q in range(8):
                        nc.tensor.matmul(pb[:], tT_sb[:, 2 * q:2 * q + 2, msl],
                                         gbc_sb[:, 2 * q:2 * q + 2, nsl],
                                         start=(q == 0), stop=(q == 7),
                                         perf_mode=mybir.MatmulPerfMode.DoubleRow)
                    to = tout_p.tile([128, 512], F32)
                    pair = n * 4 + m
                    nc.vector.scalar_tensor_tensor(
                        out=to[:], in0=a_tiles[pair][:], scalar=1.0, in1=pb[:],
                        op0=AL.mult, op1=AL.mult,
                        accum_out=ocols[:, pair:pair + 1])

            nc.sync.dma_start(out=ocols_d[:], in_=ocols[:])
            nc.sync.dma_start(out=orows_d[:], in_=orows[:1, :])
            nc.sync.dma_start(out=oemb_d[:], in_=oemb[:])

    _split_waits(nc)
    return nc


def _prep_inputs(graph, embedding, prob, graph_b, embedding_b, prob_b, tran):
    G = np.asarray(graph, np.float32)
    E = np.asarray(embedding, np.float32)
    P = np.asarray(prob, np.float32).reshape(N)
    GB = np.asarray(graph_b, np.float32)
    EB = np.asarray(embedding_b, np.float32)
    PB = np.asarray(prob_b, np.float32).reshape(N)
    T = np.asarray(tran, np.float32)

    psb = np.zeros((N, 16), FP8)
    psb[:, 0] = (P * 2048.0).astype(FP8)
    ptb = np.zeros((N, 16), FP8)
    ptb[:, 0] = (PB * 2048.0).astype(FP8)
    in_maps = []
    for idx in range(NCORES):
        r, c = idx // 2, idx % 2
        rblk = slice(512 * r, 512 * r + 512)
        ccols = slice(1024 * c, 1024 * c + 1024)
        dh = slice(64 * c, 64 * c + 64)
        jsl = slice(256 * idx, 256 * idx + 256)
        f8 = lambda x: np.ascontiguousarray(x).astype(FP8)
        in_maps.append({
            "csT": f8(G[rblk, :].T),
            "tT": f8(T[rblk, :].T * TSCALE),
            "tcc": f8(T[:, ccols] * TSCALE),
            "gbc": f8(GB[:, ccols]),
            "gbT": f8(GB[jsl, :].T),
            "ebf": f8(EB[:, dh]),
            "efT": np.ascontiguousarray(E[rblk, dh].T, dtype=np.float32),
            "ebs": np.ascontiguousarray(EB[jsl, :], dtype=np.float32),
            "psb": psb,
            "ptb": ptb,
        })
    return in_maps


def _reduce(results):
    S_main = 0.0
    S_emb = 0.0
    f1 = np.zeros(N, np.float64)
    f2 = np.zeros(N, np.float64)
    r = np.zeros(N, np.float64)
    c = np.zeros(N, np.float64)
    nE = np.zeros(N, np.float64)
    nEB = np.zeros(N, np.float64)
    for idx in range(NCORES):
        rr, cc = idx // 2, idx % 2
        rblk = slice(512 * rr, 512 * rr + 512)
        jsl = slice(256 * idx, 256 * idx + 256)
        ocols = np.asarray(results[idx]["out_cols"], np.float64)
        orows = np.asarray(results[idx]["out_rows"], np.float64)[0]
        oemb = np.asarray(results[idx]["out_emb"], np.float64)
        S_main += ocols[:, 0:8].sum() / (TSCALE * TSCALE)
        S_emb += oemb.sum() / TSCALE
        f2[jsl] = orows[512:768] / 2048.0
        nEB[jsl] = ocols[:, 24:26].reshape(256)
        nE[rblk] += orows[768:1280]
        if cc == 0:
            f1[rblk] = orows[0:512] / 2048.0
            r[rblk] = orows[1280:1792] / TSCALE
            # colsum partial over rblk: j = 16*p + t
            c += ocols[:, 8:24].reshape(N) / TSCALE
    total = (
        ((f1 + 0.5 * nE) * r).sum()
        + ((f2 + 0.5 * nEB) * c).sum()
        - 2.0 * S_main
        - S_emb
    )
    return np.float32(total)


def run_spmd(in_maps, trace=False, **kw):
    if "nc" not in _cache:
        _cache["nc"] = _build()
    return bass_utils.run_bass_kernel_spmd(
        _cache["nc"], in_maps, list(range(NCORES)), trace=trace, **kw)


def kernel(graph, embedding, prob, graph_b, embedding_b, prob_b, tran,
           weights, ole_coeff, idx):
    in_maps = _prep_inputs(graph, embedding, prob, graph_b, embedding_b,
                           prob_b, tran)
    last_err = None
    for _attempt in range(3):
        try:
            res = run_spmd(in_maps)
            return _reduce(res.results)
        except Exception as e:  # transient NRT device errors seen under axon
            last_err = e
    raise last_err

